# revision 19
# baseline (speedup 1.0000x reference)
"""Guided channel-wise 3x3 conv (per-pixel weights) on 8 Trainium2 cores.

out[b,c,h,w] = sum_{dh,dw in {-1,0,1}} input[b,c,h+dh,w+dw] * weights[b,c,k(dh,dw),h,w]
with SAME zero padding.  Shapes: input (8,64,128,128) f32,
weights (8,64,9,128,128) f32 -> out (8,64,128,128) f32.

Sharding: pure data parallelism, one batch sample per NeuronCore (B=8 cores).

Per-core layout: 128 SBUF partitions = (half, c) with p = half*64 + c; each
partition holds one 64-row half of one channel plane (input padded 66x130).
Everything on-chip is fp16: halves HBM traffic (memory-bound regime) and
doubles DVE throughput (2x_1p mode needs 2-byte packed operands); rel err
~6e-4 vs the 2e-2 gate.

The DVE does 9 multiplies + 8 accumulates (~75 us of engine time); the
pipeline hides DMA behind it:
  - taps 0-2 are processed in 16-row quarters gated on quarter-granular
    weight DMAs, so compute starts as soon as the first ~1 MB lands and the
    serial fill (input + first taps) is overlapped,
  - taps 3-8 stream whole planes through 4 buffer slots (DMA runs ahead),
  - the final accumulate runs in quarters, each immediately flushed to HBM.

DMA completions are OUT OF ORDER on this hardware (queue packets fan out
over 16 DMA engines), so a single cumulative DMA semaphore is unsound.
Every awaited transfer group gets a private semaphore, and consumers wait
for that semaphore's full count — correct under any completion order.
"""

import numpy as np

from concourse import bass, mybir
from concourse.bass_utils import run_bass_kernel_spmd

B, CI, H, W = 8, 64, 128, 128
K = 9
HH = H // 2  # rows per half-plane (64)
PR = HH + 2  # padded rows per partition (66)
PC = W + 2  # padded cols (130)
NP = 128  # SBUF partitions
FP = HH * W  # free elems per partition of one output half-plane (8192)
QF = FP // 4  # quarter free elems (2048)
QR = HH // 4  # quarter rows (16)

F16 = mybir.dt.float16

TAPS = [4, 0, 1, 2, 3, 5, 6, 7, 8]  # center tap first: it initializes out
NSLOT = 4
NQTAP = 3  # taps processed in quarters (0..NQTAP-1)

# input DMA pieces: padded row ranges (disjoint); piece q covers the rows
# needed by quarter q of any tap (16q+dh .. 16q+16+dh, dh<=2)
IN_PIECES = [(0, 18), (18, 34), (34, 50), (50, 66)]


def build_bass():
    nc = bass.Bass()
    inp = nc.declare_dram_parameter("input", [NP, PR * PC], F16, isOutput=False)
    wts = nc.declare_dram_parameter("weights", [K, NP, FP], F16, isOutput=False)
    out = nc.declare_dram_parameter("out", [NP, FP], F16, isOutput=True)

    from contextlib import ExitStack

    with ExitStack() as ctx:
        in_pad = ctx.enter_context(nc.sbuf_tensor("in_pad", [NP, PR * PC], F16))
        wt = [
            ctx.enter_context(nc.sbuf_tensor(f"wt{i}", [NP, FP], F16))
            for i in range(NSLOT)
        ]
        tmp = ctx.enter_context(nc.sbuf_tensor("tmp", [NP, FP], F16))
        out_t = ctx.enter_context(nc.sbuf_tensor("out_t", [NP, FP], F16))
        block = ctx.enter_context(nc.Block())
        in_sems = [
            ctx.enter_context(nc.semaphore(f"in_sem{q}")) for q in range(4)
        ]
        # private per-tap weight sems; quartered int8 taps get one per quarter
        wq_sems = {
            (j, q): ctx.enter_context(nc.semaphore(f"w{j}q{q}_sem"))
            for j in range(NQTAP)
            for q in range(4)
        }
        w_sems = {
            j: ctx.enter_context(nc.semaphore(f"w{j}_sem"))
            for j in range(NQTAP, K)
        }
        dve_sem = ctx.enter_context(nc.semaphore("dve_sem"))
        out_sem = ctx.enter_context(nc.semaphore("out_sem"))

        in3 = in_pad[:].rearrange("p (r w) -> p r w", r=PR)
        out3 = out_t[:].rearrange("p (r w) -> p r w", r=HH)
        tmp3 = tmp[:].rearrange("p (r w) -> p r w", r=HH)

        # dve_sem: +1 after the LAST weight-read (final mult) of each tap
        # (slot-reuse gate), then +1 per final-accumulate quarter (out gate).

        @block.scalar
        def _(scalar):
            # input pieces on the Act engine's DGE queue: they stream in
            # parallel with the SP weight queue, shortening the serial fill
            for q in range(4):
                r0, r1 = IN_PIECES[q]
                scalar.dma_start(
                    out=in_pad[:, r0 * PC : r1 * PC],
                    in_=inp[:, r0 * PC : r1 * PC],
                ).then_inc(in_sems[q], 16)

        @block.sync
        def _(sync):
            for q in range(4):
                sync.dma_start(
                    out=wt[0][:, q * QF : (q + 1) * QF],
                    in_=wts[TAPS[0], :, q * QF : (q + 1) * QF],
                ).then_inc(wq_sems[(0, q)], 16)
            for j in range(1, NQTAP):
                for q in range(4):
                    sync.dma_start(
                        out=wt[j][:, q * QF : (q + 1) * QF],
                        in_=wts[TAPS[j], :, q * QF : (q + 1) * QF],
                    ).then_inc(wq_sems[(j, q)], 16)
            for j in range(NQTAP, K):
                if j >= NSLOT:
                    sync.wait_ge(dve_sem, j - NSLOT + 1)
                sync.dma_start(out=wt[j % NSLOT][:], in_=wts[TAPS[j]]).then_inc(
                    w_sems[j], 16
                )
            for q in range(8):
                sync.wait_ge(dve_sem, K + q + 1)
                sync.dma_start(
                    out=out[:, q * (QF // 2) : (q + 1) * (QF // 2)],
                    in_=out_t[:, q * (QF // 2) : (q + 1) * (QF // 2)],
                ).then_inc(out_sem, 16)
            sync.wait_ge(out_sem, 128)

        @block.vector
        def _(vector):
            for j in range(K):
                k = TAPS[j]
                dh, dw = k // 3, k % 3
                wt3 = wt[j % NSLOT][:].rearrange("p (r w) -> p r w", r=HH)
                if j < NQTAP:
                    # quarter-granular: mult (and for j>0 accumulate) per 16 rows
                    for q in range(4):
                        if j == 0:
                            vector.wait_ge(in_sems[q], 16)
                        vector.wait_ge(wq_sems[(j, q)], 16)
                        r = q * QR
                        i0 = in3[:, r + dh : r + dh + QR, dw : dw + W]
                        if j == 0:
                            mm = vector.tensor_tensor(
                                out=out3[:, r : r + QR],
                                in0=i0,
                                in1=wt3[:, r : r + QR],
                                op=mybir.AluOpType.mult,
                            )
                        else:
                            mm = vector.tensor_tensor(
                                out=tmp3[:, r : r + QR],
                                in0=i0,
                                in1=wt3[:, r : r + QR],
                                op=mybir.AluOpType.mult,
                            )
                        if q == 3:
                            mm.then_inc(dve_sem, 1)
                        if j > 0:
                            vector.tensor_tensor(
                                out=out3[:, r : r + QR],
                                in0=out3[:, r : r + QR],
                                in1=tmp3[:, r : r + QR],
                                op=mybir.AluOpType.add,
                            )
                    continue
                vector.wait_ge(w_sems[j], 16)
                vector.tensor_tensor(
                    out=tmp3,
                    in0=in3[:, dh : dh + HH, dw : dw + W],
                    in1=wt3,
                    op=mybir.AluOpType.mult,
                ).then_inc(dve_sem, 1)
                if j == K - 1:
                    # final accumulate in eighths; each releases an out DMA
                    for q in range(8):
                        r = q * (QR // 2)
                        vector.tensor_tensor(
                            out=out3[:, r : r + QR // 2],
                            in0=out3[:, r : r + QR // 2],
                            in1=tmp3[:, r : r + QR // 2],
                            op=mybir.AluOpType.add,
                        ).then_inc(dve_sem, 1)
                else:
                    vector.tensor_tensor(
                        out=out3, in0=out3, in1=tmp3, op=mybir.AluOpType.add
                    )

    return nc


def _prep_input(x):
    """(64,128,128) f32 -> (128, 66*130) fp16 per-partition padded layout."""
    pad = np.zeros((CI, H + 2, W + 2), dtype=np.float16)
    pad[:, 1 : H + 1, 1 : W + 1] = x.astype(np.float16)
    win = np.stack([pad[:, 0:PR, :], pad[:, HH : HH + PR, :]], axis=0)
    return np.ascontiguousarray(win.reshape(NP, PR * PC))


def _prep_weights(w):
    """(64,9,128,128) f32 -> (9, 128, 64*128) fp16, partition p = half*64 + c."""
    wr = w.astype(np.float16).reshape(CI, K, 2, HH, W).transpose(1, 2, 0, 3, 4)
    return np.ascontiguousarray(wr.reshape(K, NP, FP))


def _unprep_out(o):
    """(128, 64*128) fp16 -> (64,128,128) f32."""
    return np.ascontiguousarray(
        o.astype(np.float32).reshape(2, CI, HH, W).transpose(1, 0, 2, 3).reshape(CI, H, W)
    )


_NC = None


def _get_nc():
    global _NC
    if _NC is None:
        _NC = build_bass()
    return _NC


def make_in_maps(input, weights):
    input = np.asarray(input, dtype=np.float32)
    weights = np.asarray(weights, dtype=np.float32)
    return [
        {"input": _prep_input(input[b]), "weights": _prep_weights(weights[b])}
        for b in range(B)
    ]


def kernel(input, weights):
    nc = _get_nc()
    in_maps = make_in_maps(input, weights)
    res = run_bass_kernel_spmd(nc, in_maps, list(range(B)))
    return np.stack([_unprep_out(res.results[b]["out"]) for b in range(B)], axis=0)


# revision 20
# speedup vs baseline: 1.0000x; 1.0000x over previous
"""Guided channel-wise 3x3 conv (per-pixel weights) on 8 Trainium2 cores.

out[b,c,h,w] = sum_{dh,dw in {-1,0,1}} input[b,c,h+dh,w+dw] * weights[b,c,k(dh,dw),h,w]
with SAME zero padding.  Shapes: input (8,64,128,128) f32,
weights (8,64,9,128,128) f32 -> out (8,64,128,128) f32.

Sharding: pure data parallelism, one batch sample per NeuronCore (B=8 cores).

Per-core layout: 128 SBUF partitions = (half, c) with p = half*64 + c; each
partition holds one 64-row half of one channel plane (input padded 66x130).
Everything on-chip is fp16: halves HBM traffic (memory-bound regime) and
doubles DVE throughput (2x_1p mode needs 2-byte packed operands); rel err
~6e-4 vs the 2e-2 gate.

The DVE does 9 multiplies + 8 accumulates (~75 us of engine time); the
pipeline hides DMA behind it:
  - taps 0-2 are processed in 16-row quarters gated on quarter-granular
    weight DMAs, so compute starts as soon as the first ~1 MB lands and the
    serial fill (input + first taps) is overlapped,
  - taps 3-8 stream whole planes through 4 buffer slots (DMA runs ahead),
  - the final accumulate runs in quarters, each immediately flushed to HBM.

DMA completions are OUT OF ORDER on this hardware (queue packets fan out
over 16 DMA engines), so a single cumulative DMA semaphore is unsound.
Every awaited transfer group gets a private semaphore, and consumers wait
for that semaphore's full count — correct under any completion order.
"""

import numpy as np

from concourse import bass, mybir
from concourse.bass_utils import run_bass_kernel_spmd

B, CI, H, W = 8, 64, 128, 128
K = 9
HH = H // 2  # rows per half-plane (64)
PR = HH + 2  # padded rows per partition (66)
PC = W + 2  # padded cols (130)
NP = 128  # SBUF partitions
FP = HH * W  # free elems per partition of one output half-plane (8192)
QF = FP // 4  # quarter free elems (2048)
QR = HH // 4  # quarter rows (16)

F16 = mybir.dt.float16

TAPS = [4, 0, 1, 2, 3, 5, 6, 7, 8]  # center tap first: it initializes out
NSLOT = 4
NQTAP = 3  # taps processed in quarters (0..NQTAP-1)

# input DMA pieces: padded row ranges (disjoint); piece q covers the rows
# needed by quarter q of any tap (16q+dh .. 16q+16+dh, dh<=2)
IN_PIECES = [(0, 18), (18, 34), (34, 50), (50, 66)]


def build_bass():
    nc = bass.Bass()
    inp = nc.declare_dram_parameter("input", [NP, PR * PC], F16, isOutput=False)
    wts = nc.declare_dram_parameter("weights", [K, NP, FP], F16, isOutput=False)
    out = nc.declare_dram_parameter("out", [NP, FP], F16, isOutput=True)

    from contextlib import ExitStack

    with ExitStack() as ctx:
        in_pad = ctx.enter_context(nc.sbuf_tensor("in_pad", [NP, PR * PC], F16))
        wt = [
            ctx.enter_context(nc.sbuf_tensor(f"wt{i}", [NP, FP], F16))
            for i in range(NSLOT)
        ]
        tmp = ctx.enter_context(nc.sbuf_tensor("tmp", [NP, FP], F16))
        out_t = ctx.enter_context(nc.sbuf_tensor("out_t", [NP, FP], F16))
        block = ctx.enter_context(nc.Block())
        in_sems = [
            ctx.enter_context(nc.semaphore(f"in_sem{q}")) for q in range(4)
        ]
        # private per-tap weight sems; quartered int8 taps get one per quarter
        wq_sems = {
            (j, q): ctx.enter_context(nc.semaphore(f"w{j}q{q}_sem"))
            for j in range(NQTAP)
            for q in range(4)
        }
        w_sems = {
            j: ctx.enter_context(nc.semaphore(f"w{j}_sem"))
            for j in range(NQTAP, K)
        }
        dve_sem = ctx.enter_context(nc.semaphore("dve_sem"))
        out_sem = ctx.enter_context(nc.semaphore("out_sem"))

        in3 = in_pad[:].rearrange("p (r w) -> p r w", r=PR)
        out3 = out_t[:].rearrange("p (r w) -> p r w", r=HH)
        tmp3 = tmp[:].rearrange("p (r w) -> p r w", r=HH)

        # dve_sem: +1 after the LAST weight-read (final mult) of each tap
        # (slot-reuse gate), then +1 per final-accumulate quarter (out gate).

        @block.sync
        def _(sync):
            # interleave input pieces with tap-0 weight quarters
            for q in range(4):
                r0, r1 = IN_PIECES[q]
                sync.dma_start(
                    out=in_pad[:, r0 * PC : r1 * PC],
                    in_=inp[:, r0 * PC : r1 * PC],
                ).then_inc(in_sems[q], 16)
                sync.dma_start(
                    out=wt[0][:, q * QF : (q + 1) * QF],
                    in_=wts[TAPS[0], :, q * QF : (q + 1) * QF],
                ).then_inc(wq_sems[(0, q)], 16)
            for j in range(1, NQTAP):
                for q in range(4):
                    sync.dma_start(
                        out=wt[j][:, q * QF : (q + 1) * QF],
                        in_=wts[TAPS[j], :, q * QF : (q + 1) * QF],
                    ).then_inc(wq_sems[(j, q)], 16)
            for j in range(NQTAP, K):
                if j >= NSLOT:
                    sync.wait_ge(dve_sem, j - NSLOT + 1)
                sync.dma_start(out=wt[j % NSLOT][:], in_=wts[TAPS[j]]).then_inc(
                    w_sems[j], 16
                )
            for q in range(8):
                sync.wait_ge(dve_sem, K + q + 1)
                sync.dma_start(
                    out=out[:, q * (QF // 2) : (q + 1) * (QF // 2)],
                    in_=out_t[:, q * (QF // 2) : (q + 1) * (QF // 2)],
                ).then_inc(out_sem, 16)
            sync.wait_ge(out_sem, 128)

        @block.vector
        def _(vector):
            for j in range(K):
                k = TAPS[j]
                dh, dw = k // 3, k % 3
                wt3 = wt[j % NSLOT][:].rearrange("p (r w) -> p r w", r=HH)
                if j < NQTAP:
                    # quarter-granular: mult (and for j>0 accumulate) per 16 rows
                    for q in range(4):
                        if j == 0:
                            vector.wait_ge(in_sems[q], 16)
                        vector.wait_ge(wq_sems[(j, q)], 16)
                        r = q * QR
                        i0 = in3[:, r + dh : r + dh + QR, dw : dw + W]
                        if j == 0:
                            mm = vector.tensor_tensor(
                                out=out3[:, r : r + QR],
                                in0=i0,
                                in1=wt3[:, r : r + QR],
                                op=mybir.AluOpType.mult,
                            )
                        else:
                            mm = vector.tensor_tensor(
                                out=tmp3[:, r : r + QR],
                                in0=i0,
                                in1=wt3[:, r : r + QR],
                                op=mybir.AluOpType.mult,
                            )
                        if q == 3:
                            mm.then_inc(dve_sem, 1)
                        if j > 0:
                            vector.tensor_tensor(
                                out=out3[:, r : r + QR],
                                in0=out3[:, r : r + QR],
                                in1=tmp3[:, r : r + QR],
                                op=mybir.AluOpType.add,
                            )
                    continue
                vector.wait_ge(w_sems[j], 16)
                vector.tensor_tensor(
                    out=tmp3,
                    in0=in3[:, dh : dh + HH, dw : dw + W],
                    in1=wt3,
                    op=mybir.AluOpType.mult,
                ).then_inc(dve_sem, 1)
                if j == K - 1:
                    # final accumulate in eighths; each releases an out DMA
                    for q in range(8):
                        r = q * (QR // 2)
                        vector.tensor_tensor(
                            out=out3[:, r : r + QR // 2],
                            in0=out3[:, r : r + QR // 2],
                            in1=tmp3[:, r : r + QR // 2],
                            op=mybir.AluOpType.add,
                        ).then_inc(dve_sem, 1)
                else:
                    vector.tensor_tensor(
                        out=out3, in0=out3, in1=tmp3, op=mybir.AluOpType.add
                    )

    return nc


def _prep_input(x):
    """(64,128,128) f32 -> (128, 66*130) fp16 per-partition padded layout."""
    pad = np.zeros((CI, H + 2, W + 2), dtype=np.float16)
    pad[:, 1 : H + 1, 1 : W + 1] = x.astype(np.float16)
    win = np.stack([pad[:, 0:PR, :], pad[:, HH : HH + PR, :]], axis=0)
    return np.ascontiguousarray(win.reshape(NP, PR * PC))


def _prep_weights(w):
    """(64,9,128,128) f32 -> (9, 128, 64*128) fp16, partition p = half*64 + c."""
    wr = w.astype(np.float16).reshape(CI, K, 2, HH, W).transpose(1, 2, 0, 3, 4)
    return np.ascontiguousarray(wr.reshape(K, NP, FP))


def _unprep_out(o):
    """(128, 64*128) fp16 -> (64,128,128) f32."""
    return np.ascontiguousarray(
        o.astype(np.float32).reshape(2, CI, HH, W).transpose(1, 0, 2, 3).reshape(CI, H, W)
    )


_NC = None


def _get_nc():
    global _NC
    if _NC is None:
        _NC = build_bass()
    return _NC


def make_in_maps(input, weights):
    input = np.asarray(input, dtype=np.float32)
    weights = np.asarray(weights, dtype=np.float32)
    return [
        {"input": _prep_input(input[b]), "weights": _prep_weights(weights[b])}
        for b in range(B)
    ]


def kernel(input, weights):
    nc = _get_nc()
    in_maps = make_in_maps(input, weights)
    res = run_bass_kernel_spmd(nc, in_maps, list(range(B)))
    return np.stack([_unprep_out(res.results[b]["out"]) for b in range(B)], axis=0)


# revision 22
# speedup vs baseline: 1.1098x; 1.1098x over previous
"""Guided channel-wise 3x3 conv (per-pixel weights) on 8 Trainium2 cores.

out[b,c,h,w] = sum_{dh,dw in {-1,0,1}} input[b,c,h+dh,w+dw] * weights[b,c,k(dh,dw),h,w]
with SAME zero padding.  Shapes: input (8,64,128,128) f32,
weights (8,64,9,128,128) f32 -> out (8,64,128,128) f32.

Sharding: pure data parallelism, one batch sample per NeuronCore (B=8 cores).

Per-core layout: 128 SBUF partitions = (half, c) with p = half*64 + c; each
partition holds one 64-row half of one channel plane (input padded 66x130).
Everything on-chip is fp16: halves HBM traffic (memory-bound regime) and
doubles DVE throughput (2x_1p mode needs 2-byte packed operands); rel err
~6e-4 vs the 2e-2 gate.

The DVE does 9 multiplies + 8 accumulates (~75 us of engine time); the
pipeline hides DMA behind it:
  - taps 0-2 are processed in 16-row quarters gated on quarter-granular
    weight DMAs, so compute starts as soon as the first ~1 MB lands and the
    serial fill (input + first taps) is overlapped,
  - taps 3-8 stream whole planes through 4 buffer slots (DMA runs ahead),
  - the final accumulate runs in quarters, each immediately flushed to HBM.

DMA completions are OUT OF ORDER on this hardware (queue packets fan out
over 16 DMA engines), so a single cumulative DMA semaphore is unsound.
Every awaited transfer group gets a private semaphore, and consumers wait
for that semaphore's full count — correct under any completion order.
"""

import numpy as np

from concourse import bass, mybir
from concourse.bass_utils import run_bass_kernel_spmd

B, CI, H, W = 8, 64, 128, 128
K = 9
HH = H // 2  # rows per half-plane (64)
PR = HH + 2  # padded rows per partition (66)
PC = W + 2  # padded cols (130)
NP = 128  # SBUF partitions
FP = HH * W  # free elems per partition of one output half-plane (8192)
QF = FP // 4  # quarter free elems (2048)
QR = HH // 4  # quarter rows (16)

F16 = mybir.dt.float16

TAPS = [4, 0, 1, 2, 3, 5, 6, 7, 8]  # center tap first: it initializes out
NSLOT = 4
NQTAP = 3  # taps processed in quarters (0..NQTAP-1)

# input DMA pieces: padded row ranges (disjoint); piece q covers the rows
# needed by quarter q of any tap (16q+dh .. 16q+16+dh, dh<=2)
IN_PIECES = [(0, 18), (18, 34), (34, 50), (50, 66)]


def build_bass():
    nc = bass.Bass()
    inp = nc.declare_dram_parameter("input", [NP, PR * PC], F16, isOutput=False)
    wts = nc.declare_dram_parameter("weights", [K, NP, FP], F16, isOutput=False)
    eyed = nc.declare_dram_parameter("eye", [NP, NP], F16, isOutput=False)
    out = nc.declare_dram_parameter("out", [NP, FP], F16, isOutput=True)

    from contextlib import ExitStack

    with ExitStack() as ctx:
        in_pad = ctx.enter_context(nc.sbuf_tensor("in_pad", [NP, PR * PC], F16))
        wt = [
            ctx.enter_context(nc.sbuf_tensor(f"wt{i}", [NP, FP], F16))
            for i in range(NSLOT)
        ]
        pb = [
            ctx.enter_context(nc.sbuf_tensor(f"pb{i}", [NP, FP], F16))
            for i in range(3)
        ]
        out_t = ctx.enter_context(nc.sbuf_tensor("out_t", [NP, FP], F16))
        eye_sb = ctx.enter_context(nc.sbuf_tensor("eye_sb", [NP, NP], F16))
        ps = [
            ctx.enter_context(nc.psum_tensor(f"ps{i}", [NP, QF], mybir.dt.float32))
            for i in range(2)
        ]
        block = ctx.enter_context(nc.Block())
        in_sems = [
            ctx.enter_context(nc.semaphore(f"in_sem{q}")) for q in range(4)
        ]
        # private per-tap weight sems; quartered int8 taps get one per quarter
        wq_sems = {
            (j, q): ctx.enter_context(nc.semaphore(f"w{j}q{q}_sem"))
            for j in range(NQTAP)
            for q in range(4)
        }
        w_sems = {
            j: ctx.enter_context(nc.semaphore(f"w{j}_sem"))
            for j in range(NQTAP, K)
        }
        eye_sem = ctx.enter_context(nc.semaphore("eye_sem"))
        mm_sem = ctx.enter_context(nc.semaphore("mm_sem"))
        evac_sem = ctx.enter_context(nc.semaphore("evac_sem"))
        dve_sem = ctx.enter_context(nc.semaphore("dve_sem"))
        out_sem = ctx.enter_context(nc.semaphore("out_sem"))

        in3 = in_pad[:].rearrange("p (r w) -> p r w", r=PR)
        out3 = out_t[:].rearrange("p (r w) -> p r w", r=HH)
        pb3 = [p[:].rearrange("p (r w) -> p r w", r=HH) for p in pb]

        # dve_sem: +1 after the LAST weight-read (final mult) of each tap
        # (slot-reuse gate), then +1 per final-accumulate quarter (out gate).

        @block.sync
        def _(sync):
            sync.dma_start(out=eye_sb[:], in_=eyed[:]).then_inc(eye_sem, 16)
            # interleave input pieces with tap-0 weight quarters
            for q in range(4):
                r0, r1 = IN_PIECES[q]
                sync.dma_start(
                    out=in_pad[:, r0 * PC : r1 * PC],
                    in_=inp[:, r0 * PC : r1 * PC],
                ).then_inc(in_sems[q], 16)
                sync.dma_start(
                    out=wt[0][:, q * QF : (q + 1) * QF],
                    in_=wts[TAPS[0], :, q * QF : (q + 1) * QF],
                ).then_inc(wq_sems[(0, q)], 16)
            for j in range(1, NQTAP):
                for q in range(4):
                    sync.dma_start(
                        out=wt[j][:, q * QF : (q + 1) * QF],
                        in_=wts[TAPS[j], :, q * QF : (q + 1) * QF],
                    ).then_inc(wq_sems[(j, q)], 16)
            for j in range(NQTAP, K):
                if j >= NSLOT:
                    sync.wait_ge(dve_sem, j - 2)
                sync.dma_start(out=wt[j % NSLOT][:], in_=wts[TAPS[j]]).then_inc(
                    w_sems[j], 16
                )
            # final-add order: regions 4..7 (DVE-owned chunks) then 0..3
            for i, q in enumerate([4, 5, 6, 7, 0, 1, 2, 3]):
                sync.wait_ge(dve_sem, K + i + 2)
                sync.dma_start(
                    out=out[:, q * (QF // 2) : (q + 1) * (QF // 2)],
                    in_=out_t[:, q * (QF // 2) : (q + 1) * (QF // 2)],
                ).then_inc(out_sem, 16)
            sync.wait_ge(out_sem, 128)

        @block.vector
        def _(vector):
            # dve_sem: +1 after tap0's q1 mult (PE half ready), +1 after each
            # tap's final mult (slot/product ready), +1 per final-add eighth.
            for j in range(K):
                k = TAPS[j]
                dh, dw = k // 3, k % 3
                wt3 = wt[j % NSLOT][:].rearrange("p (r w) -> p r w", r=HH)
                dst3 = pb3[j % 3]
                if j < NQTAP:
                    for q in range(4):
                        if j == 0:
                            vector.wait_ge(in_sems[q], 16)
                        vector.wait_ge(wq_sems[(j, q)], 16)
                        r = q * QR
                        i0ap = in3[:, r + dh : r + dh + QR, dw : dw + W]
                        # tap0 writes chunks 2,3 directly into out_t (initializer)
                        o = out3 if (j == 0 and q >= 2) else dst3
                        mm = vector.tensor_tensor(
                            out=o[:, r : r + QR],
                            in0=i0ap,
                            in1=wt3[:, r : r + QR],
                            op=mybir.AluOpType.mult,
                        )
                        if (j == 0 and q == 1) or q == 3:
                            mm.then_inc(dve_sem, 1)
                    if j > 0:
                        # accumulate DVE-owned chunks 2,3
                        vector.tensor_tensor(
                            out=out3[:, HH // 2 : HH],
                            in0=out3[:, HH // 2 : HH],
                            in1=dst3[:, HH // 2 : HH],
                            op=mybir.AluOpType.add,
                        )
                    continue
                vector.wait_ge(w_sems[j], 16)
                if j >= 3:
                    # product buffer j%3 must be drained by PE group j-3
                    vector.wait_ge(mm_sem, j - 2)
                vector.tensor_tensor(
                    out=dst3,
                    in0=in3[:, dh : dh + HH, dw : dw + W],
                    in1=wt3,
                    op=mybir.AluOpType.mult,
                ).then_inc(dve_sem, 1)
                if j == K - 1:
                    # final adds in eighths: DVE-owned regions 4..7 first,
                    # then PE-owned regions 0..3 once Act has evacuated PSUM
                    for i, reg in enumerate([4, 5, 6, 7, 0, 1, 2, 3]):
                        if reg == 0:
                            vector.wait_ge(evac_sem, 1)
                        if reg == 2:
                            vector.wait_ge(evac_sem, 2)
                        r = reg * (QR // 2)
                        vector.tensor_tensor(
                            out=out3[:, r : r + QR // 2],
                            in0=out3[:, r : r + QR // 2],
                            in1=dst3[:, r : r + QR // 2],
                            op=mybir.AluOpType.add,
                        ).then_inc(dve_sem, 1)
                else:
                    vector.tensor_tensor(
                        out=out3[:, HH // 2 : HH],
                        in0=out3[:, HH // 2 : HH],
                        in1=dst3[:, HH // 2 : HH],
                        op=mybir.AluOpType.add,
                    )

        @block.tensor
        def _(tensor):
            # accumulate taps 0..7 for chunks 0,1 via identity matmuls in PSUM
            tensor.wait_ge(eye_sem, 16)
            for j in range(K - 1):
                tensor.wait_ge(dve_sem, 1 if j == 0 else j + 2)
                for c in (0, 1):
                    for s in range(4):
                        mm = tensor.matmul(
                            ps[c][:, s * 512 : (s + 1) * 512],
                            eye_sb[:],
                            pb[j % 3][:, c * QF + s * 512 : c * QF + (s + 1) * 512],
                            start=(j == 0),
                            stop=(j == K - 2),
                        )
                mm.then_inc(mm_sem, 1)

        @block.scalar
        def _(scalar):
            # evacuate the PE-owned half (taps 0..7 sums) to fp16 in out_t
            for c in (0, 1):
                scalar.wait_ge(mm_sem, K - 1)
                scalar.copy(
                    out_t[:, c * QF : (c + 1) * QF], ps[c][:]
                ).then_inc(evac_sem, 1)

    return nc


def _prep_input(x):
    """(64,128,128) f32 -> (128, 66*130) fp16 per-partition padded layout."""
    pad = np.zeros((CI, H + 2, W + 2), dtype=np.float16)
    pad[:, 1 : H + 1, 1 : W + 1] = x.astype(np.float16)
    win = np.stack([pad[:, 0:PR, :], pad[:, HH : HH + PR, :]], axis=0)
    return np.ascontiguousarray(win.reshape(NP, PR * PC))


def _prep_weights(w):
    """(64,9,128,128) f32 -> (9, 128, 64*128) fp16, partition p = half*64 + c."""
    wr = w.astype(np.float16).reshape(CI, K, 2, HH, W).transpose(1, 2, 0, 3, 4)
    return np.ascontiguousarray(wr.reshape(K, NP, FP))


def _unprep_out(o):
    """(128, 64*128) fp16 -> (64,128,128) f32."""
    return np.ascontiguousarray(
        o.astype(np.float32).reshape(2, CI, HH, W).transpose(1, 0, 2, 3).reshape(CI, H, W)
    )


_NC = None


def _get_nc():
    global _NC
    if _NC is None:
        _NC = build_bass()
    return _NC


def make_in_maps(input, weights):
    input = np.asarray(input, dtype=np.float32)
    weights = np.asarray(weights, dtype=np.float32)
    eye = np.ascontiguousarray(np.eye(NP, dtype=np.float16))
    return [
        {"input": _prep_input(input[b]), "weights": _prep_weights(weights[b]), "eye": eye}
        for b in range(B)
    ]


def kernel(input, weights):
    nc = _get_nc()
    in_maps = make_in_maps(input, weights)
    res = run_bass_kernel_spmd(nc, in_maps, list(range(B)))
    return np.stack([_unprep_out(res.results[b]["out"]) for b in range(B)], axis=0)
